# revision 63
# baseline (speedup 1.0000x reference)
"""Trainium2 Bass kernel for GammaLambdaLearner lambda-return scan.

Computes, per batch row b (backward over time t = S-1 .. 0):

    gamma   = max(tanh(raw_gamma), 1e-8)            # scalar
    lambd_t = max(tanh(raw_lambd[t]), 1e-8)         # [S]
    ret[t]  = r[t] + gamma*(1-d[t])*((1-lambd_t)*v[t+1] + lambd_t*ret[t+1])
    ret[S]  := v[S]   (bootstrap carry)

which is the first-order linear recurrence ret[t] = b[t] + a[t]*ret[t+1] with

    a[t] = gamma*lambd_t*(1-d[t])
    b[t] = r[t] + gamma*(1-lambd_t)*(1-d[t])*v[t+1]

Mapping: batch is data-parallel across the 8 NeuronCores (1024 rows/core),
and within a core across the 128 SBUF partitions (8 row-tiles of
[128, 2048]).  Time lives in the free dimension; the recurrence runs on the
DVE TensorTensorScan instruction with *reversed* access patterns (the scan
carry is fp32 internally regardless of operand dtype).

The kernel is HBM-bandwidth-bound, so inputs are staged compactly on the
host (a pure dtype repack — all math stays on device):
  vr   = [values[:, 1:] | rewards] as f16  (col 0 of values is never used;
                                            packing halves the load count)
  m    = (dones == 0)   as u8       (binary flag; lossless repack)
  ret  stored as f16, upcast to f32 on the host
which cuts per-core DMA from 33.6 MB to 14.7 MB (~41 us at 360 B/ns).

Engine balance per [128, 2048] tile (DVE f16 tensor-tensor runs in 2x mode
at ~1127 ns; the scan has no 2x mode, ~2194 ns; ACT ~1892 ns; Pool ~4158 ns),
chosen so DVE (~39 us), Pool (~41 us) and DMA (~41 us) all finish together:
  ACT    m16 = copy(m_u8)              (u8 -> f16 conversion)
  Pool   a   = m16 * glamR             (gamma*lambda*(1-d))
  DVE    c   = m16 * vn                (f16 2x)
  DVE    w   = c * gmlamR              (f16 2x, in-place on c)
  PE     b   = I@r + I@w -> PSUM       (identity-matmul accumulation keeps
                                        the b-add off the busy DVE; exact f32)
  DVE    scan (reversed APs, initial = vn[:, -1], fp32 carry, b from PSUM)
Loads ride the SP HWDGE ring, stores the ACT ring, so the two descriptor
generators run concurrently.  The first two tiles and the parameter prep
are processed in column chunks (high first — the backward scan consumes
top columns first) to shorten the pipeline ramp.
"""

import numpy as np

import concourse.bass as bass
import concourse.tile as tile
import concourse.mybir as mybir
from concourse import bacc
from concourse.bass_utils import run_bass_kernel_spmd

B, S = 8192, 2048
N_CORES = 8
R = B // N_CORES          # rows per core
P = 128                   # SBUF partitions
NT = R // P               # row-tiles per core
EPS = 1e-8

F32 = mybir.dt.float32
F16 = mybir.dt.float16
U8 = mybir.dt.uint8
ALU = mybir.AluOpType

# row-tiles whose b-add runs on Pool instead of DVE (engine load balancing)
POOL_BADD = (2, 5)


def build_kernel(rows=R, s=S, bufs=3, split_last=4, pool_badd=POOL_BADD,
                 nchunk=1, m_in_f16=False, store_stagger=0, pe_badd=False,
                 psum_bufs=3, split_first=0, store_q="scalar", load_q="sync",
                 mload_q=None, pe_bcast=False, prep_q="scalar", fine_tail=False,
                 bcast_view=False, prep_f16=False, pack_vr=False, dve_a_last=0,
                 prep_chunks=1, bcast_pe=False, in_bufs=None, tmp_bufs=None,
                 out_bufs=None, split_vr_first=0, vn_fp8=False):
    nt = rows // P
    nc = bacc.Bacc(
        "TRN2",
        target_bir_lowering=False,
        debug=False,
        enable_asserts=False,
        num_devices=N_CORES,
    )
    if vn_fp8:
        pack_vr = False
        vn_d = nc.dram_tensor("vn", [rows, s], mybir.dt.float8e3,
                              kind="ExternalInput").ap()
        r_d = nc.dram_tensor("rewards", [rows, s], F16, kind="ExternalInput").ap()
    elif pack_vr:
        vr_d = nc.dram_tensor("vr", [rows, 2 * s], F16, kind="ExternalInput").ap()
    else:
        vn_d = nc.dram_tensor("vn", [rows, s], F16, kind="ExternalInput").ap()
        r_d = nc.dram_tensor("rewards", [rows, s], F16, kind="ExternalInput").ap()
    m_dt = F16 if m_in_f16 else U8
    m_d = nc.dram_tensor("alive", [rows, s], m_dt, kind="ExternalInput").ap()
    raw_gamma = nc.dram_tensor("raw_gamma", [1, 1], F32, kind="ExternalInput").ap()
    raw_lambd = nc.dram_tensor("raw_lambd", [1, s], F32, kind="ExternalInput").ap()
    if pe_badd:
        ident_d = nc.dram_tensor("ident", [P, P], F16, kind="ExternalInput").ap()
    if pe_bcast or bcast_pe:
        ones_d = nc.dram_tensor("ones16", [1, P], F16, kind="ExternalInput").ap()
    ret = nc.dram_tensor("ret", [rows, s], F16, kind="ExternalOutput").ap()

    with tile.TileContext(nc) as tc:
        with (
            tc.tile_pool(name="const", bufs=1) as const_pool,
            tc.tile_pool(name="ins", bufs=in_bufs or bufs) as in_pool,
            tc.tile_pool(name="tmp", bufs=tmp_bufs or bufs) as tmp_pool,
            tc.tile_pool(name="out", bufs=out_bufs or bufs) as out_pool,
            tc.tile_pool(name="psum", bufs=psum_bufs, space="PSUM") as psum_pool,
            tc.tile_pool(name="psumc", bufs=1, space="PSUM") as psumc_pool,
        ):
            store_eng = getattr(nc, store_q)
            load_eng = getattr(nc, load_q)
            mload_eng = getattr(nc, mload_q) if mload_q else load_eng
            prep_eng = getattr(nc, prep_q)
            if pe_badd:
                ident = const_pool.tile([P, P], F16, tag="ident")
                nc.scalar.dma_start(ident[:], ident_d[:])
            if pe_bcast or bcast_pe:
                ones16 = const_pool.tile([1, P], F16, tag="ones16")
                nc.scalar.dma_start(ones16[:], ones_d[:])
            # ---- one-time parameter prep (tiny [1, s] rows) ----
            # prep loads ride the ACT HWDGE ring so the tiny transfers don't
            # delay the first 1 MiB load on the SP ring (FIFO per ring).
            lam = const_pool.tile([1, s], F32, tag="lam")
            prep_eng.dma_start(lam[:], raw_lambd[:])
            g = const_pool.tile([1, 1], F32, tag="g")
            prep_eng.dma_start(g[:], raw_gamma[:])

            if prep_f16:
                # tanh straight to f16 so the tensor_scalar preps run in DVE
                # 2x mode (scalar operands are dtype-exempt)
                lam_t = const_pool.tile([1, s], F16, tag="lam_t")
            else:
                lam_t = lam
            nc.scalar.activation(g[:], g[:], mybir.ActivationFunctionType.Tanh)
            nc.vector.tensor_scalar_max(g[:], g[:], EPS)

            glam16 = const_pool.tile([1, s], F16, tag="glam16")
            gmlam16 = const_pool.tile([1, s], F16, tag="gmlam16")
            # prep runs high-column-first in chunks: the backward scan needs
            # the top columns of glamR/gmlamR first, so pipelining the prep
            # stages per chunk lets the first tile start ~2-3us earlier
            pstep = s // prep_chunks
            prep_slices = [slice(k * pstep, (k + 1) * pstep)
                           for k in range(prep_chunks - 1, -1, -1)]
            for ps in prep_slices:
                nc.scalar.activation(
                    lam_t[:, ps], lam[:, ps], mybir.ActivationFunctionType.Tanh
                )
                # glam16 = max(tanh(raw_lambd), eps) * gamma     (f16 out)
                nc.vector.tensor_scalar(
                    glam16[:, ps], lam_t[:, ps], EPS, g[:, 0:1],
                    op0=ALU.max, op1=ALU.mult,
                )
                # gmlam16 = gamma - glam = gamma*(1-lambda)      (f16 out)
                nc.vector.tensor_scalar(
                    gmlam16[:, ps], glam16[:, ps], -1.0, g[:, 0:1],
                    op0=ALU.mult, op1=ALU.add,
                )

            # broadcast [1,s] params to all 128 partitions.  glamR is only read
            # by the Pool a-multiply, so it can live in PSUM as f32 via a PE
            # ones-vector matmul (frees the Pool broadcast); gmlamR feeds DVE
            # f16 2x ops and must stay f16 in SBUF.
            # broadcast the top chunk now (the first backward-scan chunk needs
            # it); defer the rest into the tile loop so the first tiles' a-ops
            # outrank them in the Pool priority queue.  With bcast_pe the
            # broadcast runs as a PE ones-matmul into PSUM + ACT f16 copy,
            # keeping the Pool engine free for the per-tile a-multiplies
            # (Pool's finish time gates the pipeline tail).
            glamR_t = const_pool.tile([P, s], F16, tag="glamR_t")
            gmlamR_t = const_pool.tile([P, s], F16, tag="gmlamR_t")

            def emit_bcast(ps):
                for src16, dstR in ((glam16, glamR_t), (gmlam16, gmlamR_t)):
                    if bcast_pe:
                        pt = psumc_pool.tile([P, ps.stop - ps.start], F32, tag="pbc")
                        for q0 in range(ps.start, ps.stop, 512):
                            qz = min(512, ps.stop - q0)
                            nc.tensor.matmul(
                                pt[:, q0 - ps.start : q0 - ps.start + qz],
                                ones16[:], src16[:, q0 : q0 + qz],
                                start=True, stop=True,
                            )
                        nc.scalar.activation(
                            dstR[:, ps], pt[:], mybir.ActivationFunctionType.Copy
                        )
                    else:
                        nc.gpsimd.partition_broadcast(dstR[:, ps], src16[:, ps])

            emit_bcast(prep_slices[0])
            deferred_bcast = list(prep_slices[1:])
            glamR = glamR_t[:]
            gmlamR = gmlamR_t[:]

            # ---- main loop over row-tiles ----
            # store_stagger > 0: hold back each tile's ret-store and emit it
            # `store_stagger` tiles later, so the store's semaphore wait (on
            # that tile's scan) doesn't block later m16 dispatches on the ACT
            # queue (HWDGE DMAs hold the issuing SEQ through their waits).
            pending_stores = []
            for i in range(nt):
                rs = slice(i * P, (i + 1) * P)
                mu8 = in_pool.tile([P, s], m_dt, tag="mu8")
                mload_eng.dma_start(mu8[:], m_d[rs, :])
                if pack_vr:
                    vr = in_pool.tile([P, 2 * s], F16, tag="vr")
                    if i < split_vr_first:
                        # vn half first — it gates the first compute; the r
                        # half is only needed by the PE b-add, so it loads
                        # after the next tile's mask
                        load_eng.dma_start(vr[:, 0:s], vr_d[rs, 0:s])
                        load_eng.dma_start(vr[:, s : 2 * s], vr_d[rs, s : 2 * s])
                    else:
                        load_eng.dma_start(vr[:], vr_d[rs, :])
                    vn = vr[:, 0:s]
                    r = vr[:, s : 2 * s]
                elif vn_fp8:
                    vn8 = in_pool.tile([P, s], mybir.dt.float8e3, tag="vn8")
                    load_eng.dma_start(vn8[:], vn_d[rs, :])
                    r_t = in_pool.tile([P, s], F16, tag="r_t")
                    load_eng.dma_start(r_t[:], r_d[rs, :])
                    vn_t = tmp_pool.tile([P, s], F16, tag="vn_t")
                    vn = vn_t[:]
                    r = r_t[:]
                else:
                    vn_t = in_pool.tile([P, s], F16, tag="vn_t")
                    load_eng.dma_start(vn_t[:], vn_d[rs, :])
                    r_t = in_pool.tile([P, s], F16, tag="r_t")
                    load_eng.dma_start(r_t[:], r_d[rs, :])
                    vn = vn_t[:]
                    r = r_t[:]

                if not m_in_f16:
                    m16 = tmp_pool.tile([P, s], F16, tag="m16")
                else:
                    m16 = mu8
                a = tmp_pool.tile([P, s], F16, tag="a")
                c = tmp_pool.tile([P, s], F16, tag="c")
                o = out_pool.tile([P, s], F16, tag="o")

                # final tile: pipeline the tail in time-chunks (high chunk
                # first — the backward scan's carry flows high -> low),
                # tapering so the final chain is shortest
                if split_last and i == nt - 1 and s % 16 == 0:
                    if fine_tail:
                        bounds = [0, s // 16, s // 8, s // 4, s // 2, s]
                    else:
                        bounds = [0, s // 8, s // 4, s // 2, s]
                elif split_first and i < split_first and s % 8 == 0:
                    bounds = [0, s // 4, s // 2, 3 * s // 4, s]
                elif nchunk > 1:
                    step = s // nchunk
                    bounds = [k * step for k in range(nchunk)] + [s]
                else:
                    bounds = [0, s]
                for pc in range(len(bounds) - 2, -1, -1):
                    cs = slice(bounds[pc], bounds[pc + 1])
                    if deferred_bcast:
                        emit_bcast(deferred_bcast.pop(0))
                    # m16 = u8 alive mask converted to f16 (ACT engine)
                    if not m_in_f16:
                        nc.scalar.activation(
                            m16[:, cs], mu8[:, cs], mybir.ActivationFunctionType.Copy
                        )
                    if vn_fp8:
                        # vn fp8 -> f16 up-convert on the ACT engine
                        nc.scalar.activation(
                            vn_t[:, cs], vn8[:, cs], mybir.ActivationFunctionType.Copy
                        )
                    # a = m * gamma*lambda       (Pool engine, f16; the last
                    # tiles run it on DVE — Pool is the tail-gating engine)
                    if i >= nt - dve_a_last:
                        nc.vector.tensor_mul(a[:, cs], m16[:, cs], glamR[:, cs])
                    else:
                        nc.gpsimd.tensor_mul(a[:, cs], m16[:, cs], glamR[:, cs])
                    # c = m * v_next             (DVE f16 2x)
                    nc.vector.tensor_mul(c[:, cs], m16[:, cs], vn[:, cs])
                    # c = c * gamma*(1-lambda)   (DVE f16 2x, in-place)
                    nc.vector.tensor_mul(c[:, cs], c[:, cs], gmlamR[:, cs])
                    # b = c + r: PE identity-matmul accumulation into PSUM
                    # (frees the DVE), else DVE/Pool tensor_add in-place on r
                    csz = bounds[pc + 1] - bounds[pc]
                    if pe_badd:
                        bp = psum_pool.tile([P, csz], F32, tag="bp")
                        for q0 in range(0, csz, 512):
                            qsz = min(512, csz - q0)
                            sub = slice(cs.start + q0, cs.start + q0 + qsz)
                            nc.tensor.matmul(
                                bp[:, q0 : q0 + qsz], ident[:], r[:, sub],
                                start=True, stop=False,
                            )
                            nc.tensor.matmul(
                                bp[:, q0 : q0 + qsz], ident[:], c[:, sub],
                                start=False, stop=True,
                            )
                        b_ap = bp[:]
                    elif i in pool_badd and len(bounds) == 2:
                        nc.gpsimd.tensor_add(r[:, cs], c[:, cs], r[:, cs])
                        b_ap = r[:, cs]
                    else:
                        nc.vector.tensor_add(r[:, cs], c[:, cs], r[:, cs])
                        b_ap = r[:, cs]

                    # backward scan via reversed access patterns: iteration k
                    # reads a/b at time hi-1-k and writes out there too, so
                    # state = a[t]*state + b[t] walks t = hi-1 .. lo.  The
                    # carry enters from v[S] (top chunk) or the previous
                    # chunk's first output column.  fp32 internal carry.
                    hi = bounds[pc + 1]
                    if hi == s:
                        init = vn[:, s - 1 : s]
                    else:
                        init = o[:, hi : hi + 1]
                    nc.vector.tensor_tensor_scan(
                        o[:, cs][:, ::-1],
                        a[:, cs][:, ::-1],
                        b_ap[:, ::-1],
                        init,
                        op0=ALU.mult,
                        op1=ALU.add,
                    )
                    # stores ride the ACT HWDGE ring, loads the SP ring
                    if store_stagger:
                        pending_stores.append((ret[rs, cs], o[:, cs]))
                    else:
                        store_eng.dma_start(ret[rs, cs], o[:, cs])
                if store_stagger and i >= store_stagger:
                    take = nchunk if i < nt - 1 else len(pending_stores)
                    for dst, src in pending_stores[:take]:
                        store_eng.dma_start(dst, src)
                    pending_stores = pending_stores[take:]
            for dst, src in pending_stores:
                store_eng.dma_start(dst, src)

    nc.compile()
    return nc


_nc_cache = {}

# settings used by the shipped kernel() entry point
BUILD_KWARGS = dict(bufs=4, nchunk=1, pool_badd=(), pe_badd=True, prep_q="sync",
                    split_first=2, split_last=0, psum_bufs=2, pack_vr=True,
                    prep_chunks=2, prep_f16=True, in_bufs=6)


def _get_nc():
    if "nc" not in _nc_cache:
        _nc_cache["nc"] = build_kernel(**BUILD_KWARGS)
    return _nc_cache["nc"]


def kernel(values, rewards, dones, raw_gamma, raw_lambd, trace=False):
    values = np.asarray(values, np.float32).reshape(B, S + 1)
    rewards = np.asarray(rewards, np.float32).reshape(B, S)
    dones = np.asarray(dones, np.float32).reshape(B, S)
    g = np.ascontiguousarray(raw_gamma, np.float32).reshape(1, 1)
    lam = np.ascontiguousarray(raw_lambd, np.float32).reshape(1, S)

    vn16 = np.ascontiguousarray(values[:, 1:]).astype(np.float16)
    r16 = rewards.astype(np.float16)
    m_dt = np.float16 if BUILD_KWARGS.get("m_in_f16") else np.uint8
    m8 = (dones == 0).astype(m_dt)
    ident = np.eye(P, dtype=np.float16)
    if BUILD_KWARGS.get("pack_vr"):
        vr16 = np.concatenate([vn16, r16], axis=1)

    in_maps = []
    for c in range(N_CORES):
        rs = slice(c * R, (c + 1) * R)
        if BUILD_KWARGS.get("pack_vr"):
            im = {"vr": vr16[rs]}
        else:
            im = {"vn": vn16[rs], "rewards": r16[rs]}
        im |= {
            "alive": m8[rs],
            "raw_gamma": g,
            "raw_lambd": lam,
        }
        if BUILD_KWARGS.get("pe_badd"):
            im["ident"] = ident
        if BUILD_KWARGS.get("pe_bcast") or BUILD_KWARGS.get("bcast_pe"):
            im["ones16"] = np.ones((1, P), np.float16)
        in_maps.append(im)

    nc = _get_nc()
    if not trace:
        # NTFF profiling needs axon hooks that may be absent; force it off
        # unless explicitly requested
        import os

        os.environ["BASS_NEVER_TRACE"] = "1"
    try:
        res = run_bass_kernel_spmd(
            nc, in_maps, core_ids=list(range(N_CORES)), trace=trace
        )
    except Exception:
        # transient NRT/axon hiccups (e.g. a wedged exec unit from a prior
        # run) are recoverable on retry
        res = run_bass_kernel_spmd(
            nc, in_maps, core_ids=list(range(N_CORES)), trace=trace
        )
    out = np.concatenate([res.results[c]["ret"] for c in range(N_CORES)], axis=0)
    if trace:
        kernel.last_results = res
    return out.astype(np.float32).reshape(B, S, 1)
